# revision 32
# baseline (speedup 1.0000x reference)
"""Trainium2 Bass kernel for causal self-attention with RoPE (v2).

Problem: y = CausalSelfAttention(x) with
  B, T, C, H = 4, 2048, 1024, 16; D = 64; RoPE base 10000; no 1/sqrt(D) scale.

Sharding (hybrid data x tensor parallel): core c -> batch b = c//2, head-half
hh = c%2 (8 heads = 4 head-pairs). Each core computes qkv for its batch over
its 8 heads' weight columns, attention for those heads, and the out-projection
against its 512 rows of W_out, producing a partial [T, C]. The host sums the
2 partials per batch and adds biases. 4x less DMA than pure head sharding.

Per-core pipeline (engine-balanced, PE kept continuously busy for HAM warmth):
  p1 (per pair): qT/kT [128, T] (2 heads on partitions) via PE matmuls; RoPE
      via t2s = raw*sin (DVE, PSUM src), rp = R^T t2s (PE; exploits
      sin[i]==sin[i+32] so R^T(q*sin) == rot_half(q)*sin), q = raw*cos + rp.
      v projected then PE-transposed to [tokens, d] bf16 chunks with a shared
      64-wide ones block: [vA 64 | ones 64 | vB 64] per 128-token chunk.
  p2 (per pair, head, half): s-chunk-outer "strip" softmax. For s-chunk si,
      S strip [128 s, <=1024 q] via K=64 fp32r matmuls, ONE exp over the strip
      (ACT, bf16 out), causal mask only on the 128-wide diagonal block (DVE),
      PV accumulation po[128, 1024] with M=128 stationary [v|ones] so rows
      carry both O (64) and the denominator replicated 64x. Strip loop is
      software-pipelined (PV of strip si-1 emitted after S of si) and p1 work
      of pair p+1 is interleaved at unit granularity so the PE never idles.
  p3: recip = exp(-ln(den)) on ACT (one table set switch, DVE reciprocal is
      8cy/elem and ACT Reciprocal is banned), normalize OT in place (bf16),
      out = OT^T @ W_out accumulated over pairs in PSUM, DMA partial out.
"""

import numpy as np
from contextlib import ExitStack

import ml_dtypes
import concourse.mybir as mybir
import concourse.tile as tile
from concourse import bacc
from concourse.bass_utils import run_bass_kernel_spmd
from concourse.masks import make_identity

# Force Exp/Ln/Copy activations to resolve to the one table set containing
# all of them (natural_log_exp_and_others). Without this, the Tile scheduler
# interleaves p3's Ln with attention Exp on the ACT queue and every
# alternation pays a ~2.7us ACT_TABLE_LOAD + drain, which also starves the
# PE long enough to drop the HAM clock gate to half speed. Set positions are
# preserved so act_func_set_id indices stay valid.
_gat_orig = bacc.get_activation_tables
_UNIFIED = "natural_log_exp_and_others"


def _gat_unified(arch):
    tabs = _gat_orig(arch)
    if _UNIFIED in tabs:
        shared = {f for f in tabs[_UNIFIED]
                  if f.name.lower() in ("exp", "ln", "copy", "identity")}
        tabs = {name: (s if name == _UNIFIED else (s - shared))
                for name, s in tabs.items()}
    return tabs


bacc.get_activation_tables = _gat_unified

F32 = mybir.dt.float32
F32R = mybir.dt.float32r
BF16 = mybir.dt.bfloat16
AF = mybir.ActivationFunctionType

C = 1024
H = 16
D = 64
N_CORES = 8
T = 2048                 # tokens per core (one batch)
NP = 4                   # head pairs per core
KC = C // 128            # 8 contraction chunks for projections
TB = T // 512            # 4 token blocks
NCH = T // 128           # 16 v chunks per pair
VW = 192                 # vsb cols per chunk: [vA 64 | ones 64 | vB 64]
ROPE_BASE = 10000.0
DEBUG = False


def _chop512(a, b):
    """Split [a, b) at 512 boundaries -> list of (lo, hi)."""
    out = []
    while a < b:
        nxt = min(b, (a // 512 + 1) * 512)
        out.append((a, nxt))
        a = nxt
    return out


def build_program(use_qk_bias):
    nc = bacc.Bacc("TRN2", target_bir_lowering=False, debug=False,
                   num_devices=N_CORES)

    xT = nc.dram_tensor("xT", [C, T], F32R, kind="ExternalInput").ap()
    wq = nc.dram_tensor("wq", [C, 512], F32R, kind="ExternalInput").ap()
    wk = nc.dram_tensor("wk", [C, 512], F32R, kind="ExternalInput").ap()
    wv = nc.dram_tensor("wv", [C, 512], F32R, kind="ExternalInput").ap()
    wo = nc.dram_tensor("wo", [512, C], BF16, kind="ExternalInput").ap()
    cosT = nc.dram_tensor("cosT", [128, T], F32, kind="ExternalInput").ap()
    sinT = nc.dram_tensor("sinT", [128, T], F32, kind="ExternalInput").ap()
    mskP = nc.dram_tensor("mskP", [128, 128], BF16, kind="ExternalInput").ap()
    rot = nc.dram_tensor("rot", [128, 128], F32R, kind="ExternalInput").ap()
    if use_qk_bias:
        bq = nc.dram_tensor("bq", [128, NP], F32, kind="ExternalInput").ap()
        bk = nc.dram_tensor("bk", [128, NP], F32, kind="ExternalInput").ap()
    outp = nc.dram_tensor("outp", [T, C], F32, kind="ExternalOutput").ap()
    if DEBUG:
        qTd = nc.dram_tensor("qTd", [128, NP * T], F32, kind="ExternalOutput").ap()
        kTd = nc.dram_tensor("kTd", [128, NP * T], F32, kind="ExternalOutput").ap()
        vsbd = nc.dram_tensor("vsbd", [128, NP * NCH * VW], F32, kind="ExternalOutput").ap()
        OTd = nc.dram_tensor("OTd", [128, NP * T], F32, kind="ExternalOutput").ap()
        densd = nc.dram_tensor("densd", [128, 8 * 1024], F32, kind="ExternalOutput").ap()
        densr = nc.dram_tensor("densr", [128, 8 * 1024], F32, kind="ExternalOutput").ap()
        bcd = nc.dram_tensor("bcd", [128, 16 * 1024], F32, kind="ExternalOutput").ap()
        OTn = nc.dram_tensor("OTn", [128, NP * T], F32, kind="ExternalOutput").ap()

    with tile.TileContext(nc) as tc:
        with ExitStack() as res:
            persist = res.enter_context(tc.tile_pool(name="persist", bufs=1))
            qT = persist.tile([128, NP * T], F32R)
            kT = persist.tile([128, NP * T], F32R)
            vsb = persist.tile([128, NP * NCH * VW], BF16)
            OT = persist.tile([128, NP * T], BF16)
            # den staging: slot (p*2+f) x 1024 cols; row 64 = head A's den,
            # row 0 = head B's den (lane-aligned copies out of po)
            dens = persist.tile([128, 8 * 1024], BF16)
            cos_sb = persist.tile([128, T], F32)
            sin_sb = persist.tile([128, T], F32)
            msk_sb = persist.tile([128, 128], BF16)
            rot_sb = persist.tile([128, 128], F32R)
            ident = persist.tile([128, 128], BF16)
            ones_sb = persist.tile([128, 128], BF16)
            nc.vector.memset(ones_sb[:], 1.0)
            shift_sb = persist.tile([128, 1], F32)
            nc.vector.memset(shift_sb[:], -15.0)
            nc.vector.memset(dens[:], 1.0)
            make_identity(nc, ident[:])
            if use_qk_bias:
                bq_sb = persist.tile([128, NP], F32)
                bk_sb = persist.tile([128, NP], F32)
                nc.sync.dma_start(bq_sb[:], bq[:])
                nc.sync.dma_start(bk_sb[:], bk[:])

            with ExitStack() as p12:
                wpool = p12.enter_context(tc.tile_pool(name="wp", bufs=2))
                xpool = p12.enter_context(tc.tile_pool(name="xp", bufs=8))
                stage = p12.enter_context(tc.tile_pool(name="st", bufs=3))
                vrawp = p12.enter_context(tc.tile_pool(name="vr", bufs=2))
                ppool = p12.enter_context(tc.tile_pool(name="pp", bufs=2))
                stripP = p12.enter_context(
                    tc.tile_pool(name="stripP", bufs=2, space="PSUM"))
                poP = p12.enter_context(
                    tc.tile_pool(name="poP", bufs=1, space="PSUM"))
                p1psum = ExitStack()
                accP = p1psum.enter_context(
                    tc.tile_pool(name="accP", bufs=2, space="PSUM"))
                rpP = p1psum.enter_context(
                    tc.tile_pool(name="rpP", bufs=2, space="PSUM"))

                def p1_units(p):
                    """Projection + rope + v for pair p. Yields per unit."""
                    wq_sb = wpool.tile([128, C], F32R, tag="wq")
                    wk_sb = wpool.tile([128, C], F32R, tag="wk")
                    wv_sb = wpool.tile([128, C], F32R, tag="wv")
                    # pair 0 fans weight DMAs out over idle engine queues so
                    # the first matmuls are not stuck behind one DGE queue
                    q_eng = nc.scalar if p == 0 else nc.sync
                    if p == 0:
                        nc.sync.dma_start(rot_sb[:], rot[:])
                    for k in range(KC):
                        ks = slice(k * 128, (k + 1) * 128)
                        ps = slice(p * 128, (p + 1) * 128)
                        q_eng.dma_start(wq_sb[:, ks], wq[ks, ps])
                    if p == 0:
                        nc.scalar.dma_start(sin_sb[:], sinT[:])
                        nc.scalar.dma_start(cos_sb[:], cosT[:])
                        nc.sync.dma_start(msk_sb[:], mskP[:])
                    for k in range(KC):
                        ks = slice(k * 128, (k + 1) * 128)
                        ps = slice(p * 128, (p + 1) * 128)
                        q_eng.dma_start(wk_sb[:, ks], wk[ks, ps])
                        q_eng.dma_start(wv_sb[:, ks], wv[ks, ps])
                    # ones blocks for this pair's v chunks
                    for ch in range(NCH):
                        cb = (p * NCH + ch) * VW
                        nc.vector.memset(vsb[:, cb + 64:cb + 128], 1.0)
                    yield
                    for tb in range(TB):
                        tsl = slice(tb * 512, (tb + 1) * 512)         # tokens
                        dsl = slice(p * T + tb * 512, p * T + (tb + 1) * 512)
                        xc = []
                        for k in range(KC):
                            t = xpool.tile([128, 512], F32R, tag="xc")
                            nc.sync.dma_start(
                                t[:], xT[k * 128:(k + 1) * 128, tsl])
                            xc.append(t)
                        for w_sb, b_name, dstT in ((wq_sb, "bq", qT),
                                                   (wk_sb, "bk", kT)):
                            acc = accP.tile([128, 512], F32, tag="acc")
                            for k in range(KC):
                                nc.tensor.matmul(
                                    acc[:], w_sb[:, k * 128:(k + 1) * 128],
                                    xc[k][:], start=(k == 0), stop=(k == KC - 1))
                            yield
                            if use_qk_bias:
                                b_sb = bq_sb if b_name == "bq" else bk_sb
                                raws = stage.tile([128, 512], F32R, tag="st")
                                nc.vector.tensor_scalar_add(
                                    raws[:], acc[:], b_sb[:, p:p + 1])
                                src = raws
                            else:
                                src = acc
                            t2s = stage.tile([128, 512], F32R, tag="st")
                            nc.vector.tensor_mul(t2s[:], src[:], sin_sb[:, tsl])
                            rp = rpP.tile([128, 512], F32, tag="rp")
                            nc.tensor.matmul(rp[:], rot_sb[:], t2s[:],
                                             start=True, stop=True)
                            t1 = stage.tile([128, 512], F32R, tag="st")
                            nc.vector.tensor_mul(t1[:], src[:], cos_sb[:, tsl])
                            nc.vector.tensor_add(dstT[:, dsl], t1[:], rp[:])
                            yield
                        vacc = accP.tile([128, 512], F32, tag="acc")
                        for k in range(KC):
                            nc.tensor.matmul(
                                vacc[:], wv_sb[:, k * 128:(k + 1) * 128],
                                xc[k][:], start=(k == 0), stop=(k == KC - 1))
                        yield
                        vraw = vrawp.tile([128, 512], BF16, tag="vr")
                        nc.vector.tensor_copy(vraw[:], vacc[:])
                        for sub in range(4):
                            tp = rpP.tile([128, 512], BF16, tag="rp")
                            nc.tensor.transpose(
                                tp[:, 0:128],
                                vraw[:, sub * 128:(sub + 1) * 128], ident[:])
                            cb = (p * NCH + tb * 4 + sub) * VW
                            nc.vector.tensor_copy(vsb[:, cb:cb + 64],
                                                  tp[:, 0:64])
                            nc.vector.tensor_copy(vsb[:, cb + 128:cb + 192],
                                                  tp[:, 64:128])
                        yield

                def p2_units(p, fsel=(0, 1)):
                    """Attention for pair p: halves in fsel, 2 heads each."""
                    for f in fsel:
                        for h in range(2):
                            hs = slice(h * 64, (h + 1) * 64)
                            base = f * 1024
                            strips = list(range(8 * f + 8))
                            po = poP.tile([128, 1024], F32, tag="po")
                            prev = None
                            for si in strips + [None]:
                                if si is not None:
                                    qlo = max(si * 128, base)
                                    pieces = _chop512(qlo, base + 1024)
                                    P = ppool.tile([128, 1024], BF16, tag="pp")
                                    cL = qlo - base
                                    for (a, b) in pieces:
                                        # per-piece S tile (2 bufs): S matmuls
                                        # never wait on the previous strip's
                                        # exp read. exp(S - SHIFT) keeps den
                                        # inside ACT Ln's valid window
                                        # (|ln x| < ~45); the shift cancels in
                                        # the normalization.
                                        S = stripP.tile([128, 512], F32,
                                                        tag="S")
                                        nc.tensor.matmul(
                                            S[:, 0:b - a],
                                            kT[hs, p * T + si * 128:
                                               p * T + si * 128 + 128],
                                            qT[hs, p * T + a:p * T + b],
                                            start=True, stop=True)
                                        nc.scalar.activation(
                                            P[:, a - base:b - base],
                                            S[:, 0:b - a], AF.Exp,
                                            bias=shift_sb[:, 0:1])
                                    if si * 128 >= base:   # diagonal block
                                        nc.vector.tensor_mul(
                                            P[:, cL:cL + 128],
                                            P[:, cL:cL + 128], msk_sb[:])
                                    cur = (si, P, pieces)
                                else:
                                    cur = None
                                if prev is not None:
                                    psi, pP, ppieces = prev
                                    vb = (p * NCH + psi) * VW + h * 64
                                    for (a, b) in ppieces:
                                        qb = a // 512
                                        nc.tensor.matmul(
                                            po[:, a - base:b - base],
                                            vsb[:, vb:vb + 128],
                                            pP[:, a - base:b - base],
                                            start=(psi == 0),
                                            stop=(psi == 4 * qb + 3))
                                prev = cur
                                yield
                            # drain: O rows on ACT, den row on DVE (in
                            # parallel, so po is reusable sooner)
                            slot = p * 2 + f
                            ssl = slice(slot * 1024, (slot + 1) * 1024)
                            nc.vector.tensor_copy(
                                OT[hs, p * T + base:p * T + base + 1024],
                                po[hs, :])
                            drow = slice(64, 65) if h == 0 else slice(0, 1)
                            nc.vector.tensor_copy(dens[drow, ssl],
                                                  po[drow, :])
                            yield

                # ---- phased emission --------------------------------
                # A: p1(0); p2(pair p) || p1(p+1) for p=0..2
                # B: close p1 PSUM pools; open bc/out pools
                # C: p2(pair3, f0); then p2(pair3, f1) with norm-f0 +
                #    out-proj-f0 interleaved as PE-dense filler
                # D: norm-f1 + out-proj-f1 tail (dense matmul runs)
                _STOP = object()

                def drain(g):
                    if g is not None:
                        for _ in g:
                            pass

                def adv(g, k):
                    for _ in range(k):
                        if next(g, _STOP) is _STOP:
                            return

                def interleave(g2, n2, g1, n1, start=0):
                    cnt = 0
                    done = 0
                    for _ in g2:
                        cnt += 1
                        if g1 is None or cnt <= start:
                            continue
                        want = min(n1, (cnt - start) * n1
                                   // max(1, n2 - start))
                        adv(g1, want - done)
                        done = want
                    drain(g1)

                drain(p1_units(0))
                for p in range(3):
                    interleave(p2_units(p), 56, p1_units(p + 1), 25)
                p1psum.close()

                bcP = p12.enter_context(
                    tc.tile_pool(name="bcP", bufs=1, space="PSUM"))
                outP = p12.enter_context(
                    tc.tile_pool(name="outP", bufs=2, space="PSUM"))
                lnp = p12.enter_context(tc.tile_pool(name="lnp", bufs=1))
                ostage = p12.enter_context(tc.tile_pool(name="os", bufs=4))
                # wo lives in the (now dead) projection-weight slots
                wo_sb1 = wpool.tile([128, 2 * C], BF16, tag="wq")
                wo_sb2 = wpool.tile([128, 2 * C], BF16, tag="wk")
                for p4 in range(NP):
                    dst = wo_sb1 if p4 < 2 else wo_sb2
                    nc.sync.dma_start(dst[:, (p4 % 2) * C:(p4 % 2 + 1) * C],
                                      wo[p4 * 128:(p4 + 1) * 128, :])

                def norm_units(slots):
                    # 1/den = exp(-ln(den)) (both live in the unified ACT
                    # table set, so this is safe mid-stream), then bc
                    # broadcast via K=1 ones matmul and in-place normalize.
                    for slot in slots:
                        p4, f = slot // 2, slot % 2
                        ssl = slice(slot * 1024, (slot + 1) * 1024)
                        lt = lnp.tile([128, 1024], F32, tag="ln")
                        nc.scalar.activation(lt[:], dens[:, ssl], AF.Ln)
                        nc.scalar.activation(dens[:, ssl], lt[:], AF.Exp,
                                             scale=-1.0)
                        yield
                        osl = slice(p4 * T + f * 1024,
                                    p4 * T + f * 1024 + 1024)
                        for h in range(2):
                            hs = slice(h * 64, (h + 1) * 64)
                            drow = slice(64, 65) if h == 0 else slice(0, 1)
                            bc = bcP.tile([128, 1024], F32, tag="bc")
                            for n in range(2):
                                nc.tensor.matmul(
                                    bc[:, n * 512:(n + 1) * 512],
                                    ones_sb[drow, 0:128],
                                    dens[drow, slot * 1024 + n * 512:
                                         slot * 1024 + (n + 1) * 512],
                                    start=True, stop=True)
                            nc.vector.tensor_mul(OT[hs, osl], OT[hs, osl],
                                                 bc[hs, :])
                            yield

                def outproj_units(tcs):
                    for tc_i in tcs:
                        for n in range(2):
                            oacc = outP.tile([128, 512], F32, tag="oacc")
                            for p4 in range(NP):
                                nc.tensor.matmul(
                                    oacc[:],
                                    OT[:, p4 * T + tc_i * 128:
                                       p4 * T + tc_i * 128 + 128],
                                    (wo_sb1 if p4 < 2 else wo_sb2)[
                                        :, (p4 % 2) * C + n * 512:
                                        (p4 % 2) * C + n * 512 + 512],
                                    start=(p4 == 0), stop=(p4 == NP - 1))
                            osb = ostage.tile([128, 512], F32, tag="os")
                            if tc_i >= 8 and (tc_i + n) % 2 == 0:
                                nc.scalar.activation(osb[:], oacc[:], AF.Copy)
                            else:
                                nc.vector.tensor_copy(osb[:], oacc[:])
                            nc.sync.dma_start(
                                outp[tc_i * 128:(tc_i + 1) * 128,
                                     n * 512:(n + 1) * 512], osb[:])
                        if tc_i % 2 == 1:
                            yield

                from itertools import chain
                interleave(p2_units(3, fsel=(0,)), 20,
                           norm_units([0, 2, 4]), 9)
                interleave(p2_units(3, fsel=(1,)), 36,
                           chain(norm_units([6]),
                                 outproj_units(range(0, 8)),
                                 norm_units([1, 3, 5])), 16)
                drain(chain(norm_units([7]),
                            outproj_units(range(8, 16))))

    nc.compile()
    return nc


def make_rope_tables(t_len, dtype=np.float32):
    j = np.arange(32, dtype=np.float32)
    inv_freq = (1.0 / (ROPE_BASE ** (2.0 * j / D))).astype(np.float32)
    t = np.arange(t_len, dtype=np.float32)
    freqs = t[None, :] * inv_freq[:, None]          # [32, T]
    half = np.concatenate([freqs, freqs], axis=0)   # [64, T]
    cosT = np.cos(half).astype(dtype)
    sinT = np.sin(half).astype(dtype)
    return (np.concatenate([cosT, cosT], axis=0),   # [128, T] (2 heads)
            np.concatenate([sinT, sinT], axis=0))


def make_rot_matrix():
    """lhsT R [128,128] s.t. (R.T @ x)[m] = rotate_half(x)[m] per 64-row head."""
    R = np.zeros((128, 128), dtype=np.float32)
    for hb in (0, 64):
        for m in range(32):
            R[hb + m + 32, hb + m] = -1.0
            R[hb + m, hb + m + 32] = 1.0
    return R


def make_diag_mask():
    """[128,128] bf16: 1 where s_idx <= q_idx (valid), else 0."""
    m = np.triu(np.ones((128, 128), dtype=np.float32))
    return m.astype(ml_dtypes.bfloat16)


def prep_in_maps(x, W_qkv, b_qkv, W_out, B, T_, use_qk_bias, use_v_bias=None,
                 n_cores=N_CORES):
    cosT, sinT = make_rope_tables(T_)
    mskP = make_diag_mask()
    rotm = make_rot_matrix()
    xTs = [np.ascontiguousarray(x[b].T) for b in range(B)]
    in_maps = []
    for c in range(n_cores):
        b, hh = c // 2, c % 2
        cols = slice(hh * 512, (hh + 1) * 512)
        m = {
            "xT": xTs[b],
            "wq": np.ascontiguousarray(W_qkv[:, 0 * C:1 * C][:, cols]),
            "wk": np.ascontiguousarray(W_qkv[:, 1 * C:2 * C][:, cols]),
            "wv": np.ascontiguousarray(W_qkv[:, 2 * C:3 * C][:, cols]),
            "wo": np.ascontiguousarray(W_out[cols, :]).astype(
                ml_dtypes.bfloat16),
            "cosT": cosT, "sinT": sinT, "mskP": mskP, "rot": rotm,
        }
        if use_qk_bias:
            m["bq"] = np.ascontiguousarray(
                b_qkv[0 * C:1 * C][cols]).reshape(NP, 128).T.copy()
            m["bk"] = np.ascontiguousarray(
                b_qkv[1 * C:2 * C][cols]).reshape(NP, 128).T.copy()
        in_maps.append(m)
    return in_maps


_CACHE = {}


def _get_program(key):
    if isinstance(key, tuple):
        use_qk_bias = bool(key[2]) if len(key) > 2 else False
    else:
        use_qk_bias = bool(key)
    ck = use_qk_bias
    if ck not in _CACHE:
        _CACHE[ck] = build_program(use_qk_bias)
    return _CACHE[ck]


def kernel(x, W_qkv, b_qkv, W_out, b_out):
    x = np.asarray(x, dtype=np.float32)
    W_qkv = np.asarray(W_qkv, dtype=np.float32)
    b_qkv = np.asarray(b_qkv, dtype=np.float32)
    W_out = np.asarray(W_out, dtype=np.float32)
    b_out = np.asarray(b_out, dtype=np.float32)
    B, T_, C_ = x.shape
    assert (B, T_, C_) == (4, T, C), (B, T_, C_)
    use_qk_bias = bool(np.any(b_qkv[:2 * C]))
    use_v_bias = bool(np.any(b_qkv[2 * C:]))
    nc = _get_program((B, T_, use_qk_bias, use_v_bias))
    in_maps = prep_in_maps(x, W_qkv, b_qkv, W_out, B, T_, use_qk_bias)
    res = run_bass_kernel_spmd(nc, in_maps, list(range(N_CORES)))
    y = np.empty((B, T_, C_), dtype=np.float32)
    for b in range(B):
        y[b] = res.results[2 * b]["outp"]
        y[b] += res.results[2 * b + 1]["outp"]
    extra = b_out.astype(np.float64)
    if use_v_bias:
        extra = extra + b_qkv[2 * C:].astype(np.float64) @ W_out.astype(
            np.float64)
    y += extra.astype(np.float32)[None, None, :]
    return y
